# revision 19
# baseline (speedup 1.0000x reference)
"""BitConv2d forward on 8 Trainium2 NeuronCores (SPMD data-parallel).

Strategy:
  - Shard batch (32) -> 4 images per core; replicate the tiny bit-plane
    weights/scales on every core. No collectives needed (forward only).
  - On each core, reconstruct the integer weight planes on device:
        W_int[o,i,kh,kw] = sum_b (pweight-nweight)[...,b] * 2^(3-b)   (exact, in [-15,15])
    and fold scale/15 plus the bias into the PSUM->SBUF epilogue.
  - Everything on the x-side runs in bf16: the input is cast f32->bf16 during
    the SWDGE DMA (weights are small integers, exact in bf16; x keeps 8 mantissa
    bits -> ~1e-3 output error, far inside the 2e-2 gate). The output is written
    back as bf16 too, halving HBM write traffic (the DMA engines, ~205 GB/s
    aggregate, are the binding resource).
  - 3x3 same-pad conv as 9 accumulating matmuls per output tile. The image is
    row-flattened WITHOUT horizontal padding (rows contiguous at stride 112) so
    every transfer is a large contiguous block. Partitions 0:64 hold x rows
    -1..56 (top half), partitions 64:128 hold rows 55..112 (bottom half);
    block-diagonal [[W_t,0],[0,W_t]] stationaries serve both halves at once.
    Input DMAs are chunked (4 per half) so matmuls chase the incoming stream
    instead of waiting for whole images.
  - Without column padding, taps at dc=+-1 wrap across row boundaries and
    contaminate output columns 0 and 111; repaired by 6 tiny matmuls per image
    (3-tap 1-D convs over the edge columns, reusing the block-diag stationaries)
    plus a strided PSUM subtract on the vector engine.
"""

import numpy as np

B, C, H, W = 32, 64, 112, 112
NB = 4
CORES = 8
BPC = B // CORES  # images per core

HALF = H // 2            # 56 output rows per partition-group
GW = HALF * W            # 6272 output columns per group
XCOLS = 1 + W + 57 * W + 1  # guard + extra row + 57 data rows + guard = 6498
XBASE = 1                # column of x-row -1 (top) / x-row 55 (bottom)

N_TILES = [(i * 512, 512) for i in range(11)] + [(5632, 320), (5952, 320)]
TAP_OFFS = [kh * W + kw - 1 for kh in range(3) for kw in range(3)]
IN_CHUNKS = [(0, 15), (15, 14), (29, 14), (43, 14)]  # (row0, nrows) covering 57

_CACHE = {}


def _build():
    if "nc" in _CACHE:
        return _CACHE["nc"]
    import concourse.bacc as bacc
    import concourse.mybir as mybir
    from concourse import tile
    from concourse.masks import make_identity

    f32 = mybir.dt.float32
    bf16 = mybir.dt.bfloat16
    u32 = mybir.dt.uint32
    mult = mybir.AluOpType.mult
    add = mybir.AluOpType.add

    nc = bacc.Bacc("TRN2", target_bir_lowering=False, debug=False, num_devices=CORES)

    x_d = nc.dram_tensor("x", [BPC, C, H, W], f32, kind="ExternalInput").ap()
    pw_d = nc.dram_tensor("pweight", [C, C, 3, 3, NB], f32, kind="ExternalInput").ap()
    nw_d = nc.dram_tensor("nweight", [C, C, 3, 3, NB], f32, kind="ExternalInput").ap()
    sc_d = nc.dram_tensor("scale", [1], f32, kind="ExternalInput").ap()
    pb_d = nc.dram_tensor("pbias", [C, NB], f32, kind="ExternalInput").ap()
    nb_d = nc.dram_tensor("nbias", [C, NB], f32, kind="ExternalInput").ap()
    bs_d = nc.dram_tensor("biasscale", [1], f32, kind="ExternalInput").ap()
    y_d = nc.dram_tensor("y", [BPC, C, H, W], bf16, kind="ExternalOutput").ap()

    with tile.TileContext(nc) as tc:
        with (
            tc.tile_pool(name="consts", bufs=1) as consts,
            tc.tile_pool(name="xpool", bufs=2) as xpool,
            tc.tile_pool(name="opool", bufs=2) as opool,
            tc.tile_pool(name="epool", bufs=2) as epool,
            tc.tile_pool(name="pspool", bufs=5, space="PSUM") as pspool,
            tc.tile_pool(name="fixpool", bufs=1, space="PSUM") as fixpool,
            tc.tile_pool(name="psum_t", bufs=1, space="PSUM") as psum_t,
        ):
            ident = consts.tile([128, 128], bf16, tag="ident")
            lhsT = [
                consts.tile([128, 128], bf16, tag=f"lhsT{t}", name=f"lhsT{t}")
                for t in range(9)
            ]
            scale_vec = consts.tile([128, 1], f32, tag="scale_vec")
            bias_vec = consts.tile([128, 1], f32, tag="bias_vec")

            # the identity (for weight transposes) must be FIRST on the Q7 queue:
            # everything queued behind the image-chunk descriptor generation
            # arrives ~10us late otherwise
            make_identity(nc, ident[:])

            # ---- weight/bias DMAs on the fast-start HWDGE sync ring ----
            # [128, 1152] layout: partition (h,o) holds W[o, i=32h..32h+31, :, :, :]
            # (full 128 partitions -> all 16 SDMA engines)
            wp = consts.tile([128, 32 * 9 * NB], f32, tag="wp")
            wn = consts.tile([128, 32 * 9 * NB], f32, tag="wn")
            pw_f = pw_d.rearrange("o i kh kw b -> o (i kh kw b)")
            nw_f = nw_d.rearrange("o i kh kw b -> o (i kh kw b)")
            nc.sync.dma_start(wp[0:C, :], pw_f[:, 0:1152])
            nc.sync.dma_start(wp[C:128, :], pw_f[:, 1152:2304])
            nc.sync.dma_start(wn[0:C, :], nw_f[:, 0:1152])
            nc.sync.dma_start(wn[C:128, :], nw_f[:, 1152:2304])
            pbt = consts.tile([128, NB], f32, tag="pbt")
            nbt = consts.tile([128, NB], f32, tag="nbt")
            nc.sync.dma_start(pbt[0:C, :], pb_d)
            nc.sync.dma_start(pbt[C:128, :], pb_d)
            nc.sync.dma_start(nbt[0:C, :], nb_d)
            nc.sync.dma_start(nbt[C:128, :], nb_d)
            bsv = consts.tile([128, 1], f32, tag="bsv")
            nc.sync.dma_start(bsv[:], bs_d.to_broadcast((128, 1)))
            nc.sync.dma_start(scale_vec[:], sc_d.to_broadcast((128, 1)))

            # ---- image load pipeline: chunked SWDGE cast-DMAs (f32 -> bf16) ----
            # fine chunks for image 0 (compute chases the stream); coarse after
            # (Q7 descriptor-gen is ~1us per call). DMAs before memsets: the
            # ranges are disjoint, and the DMAs are the critical path.
            def load_image(b):
                xs = xpool.tile([128, XCOLS], bf16, tag="xs", name=f"xs{b}", bufs=3)
                v0 = xs[0:C, XBASE + W : XBASE + W + 57 * W].rearrange(
                    "p (r w) -> p r w", w=W
                )
                v1 = xs[C:128, XBASE : XBASE + 57 * W].rearrange(
                    "p (r w) -> p r w", w=W
                )
                chunks = IN_CHUNKS if b == 0 else [(0, 29), (29, 28)]
                for r0, nr in chunks:
                    nc.gpsimd.dma_start(v0[:, r0 : r0 + nr, :], x_d[b, :, r0 : r0 + nr, :])
                    nc.gpsimd.dma_start(
                        v1[:, r0 : r0 + nr, :], x_d[b, :, 55 + r0 : 55 + r0 + nr, :]
                    )
                nc.gpsimd.memset(xs[0:C, 0 : XBASE + W], 0)
                nc.gpsimd.memset(xs[0:C, XCOLS - 1 : XCOLS], 0)
                nc.gpsimd.memset(xs[C:128, 0:XBASE], 0)
                nc.gpsimd.memset(xs[C:128, XBASE + 57 * W : XCOLS], 0)
                return xs

            xs_next = load_image(0)
            xs_next2 = load_image(1)

            # ---- weight/bias reconstruction (overlaps image-0 DMA) ----
            # Processed in 3 transpose-groups (taps 0-3 / 4-7 / 8) so lhsT[0]
            # is ready early and the first conv matmuls start while the rest
            # of the weight prep continues.
            nc.vector.tensor_sub(wp[:], wp[:], wn[:])  # d = p - n
            # bit-combine into tap-major W_int [(h,o), (t, il)]:
            # w = ((d0*8 + d3) + d1*4) + d2*2 via scalar_tensor_tensor chains
            # (f32 in, bf16 out on the last write; integers <=15 stay exact)
            wi = consts.tile([128, 9 * 32], f32, tag="wi")
            wt2 = consts.tile([128, 9 * 32], bf16, tag="wt2")
            wi_v = wi[:].rearrange("p (t i) -> p t i", t=9)
            wt2_v = wt2[:].rearrange("p (t i) -> p t i", t=9)
            d_v = wp[:].rearrange("p (i t b) -> p t i b", t=9, b=NB)
            for t in range(9):
                nc.vector.memset(lhsT[t][:], 0)
            for g, (tp0, ntp) in enumerate([(0, 4), (4, 4), (8, 1)]):
                dv = d_v[:, tp0 : tp0 + ntp]
                wiv = wi_v[:, tp0 : tp0 + ntp]
                wtv = wt2_v[:, tp0 : tp0 + ntp]
                nc.vector.scalar_tensor_tensor(
                    out=wtv, in0=dv[:, :, :, 0], scalar=8.0, in1=dv[:, :, :, 3],
                    op0=mult, op1=add,
                )
                nc.vector.scalar_tensor_tensor(
                    out=wiv, in0=dv[:, :, :, 1], scalar=4.0, in1=wtv,
                    op0=mult, op1=add,
                )
                nc.vector.scalar_tensor_tensor(
                    out=wtv, in0=dv[:, :, :, 2], scalar=2.0, in1=wiv,
                    op0=mult, op1=add,
                )
                cols = 32 * ntp
                tps = psum_t.tile([128, 128], bf16, tag="tps", name=f"tps{g}", bufs=2)
                nc.tensor.transpose(
                    tps[0:cols, :], wt2[:, tp0 * 32 : tp0 * 32 + cols], ident[:]
                )
                for t in range(tp0, tp0 + ntp):
                    rt = (t - tp0) * 32
                    nc.scalar.copy(lhsT[t][0:32, 0:C], tps[rt : rt + 32, 0:C])
                    nc.scalar.copy(lhsT[t][32:C, 0:C], tps[rt : rt + 32, C:128])
                    nc.vector.tensor_copy(
                        lhsT[t][C : C + 32, C:128], tps[rt : rt + 32, 0:C]
                    )
                    nc.vector.tensor_copy(
                        lhsT[t][C + 32 : 128, C:128], tps[rt : rt + 32, C:128]
                    )
            # bias vector, duplicated across both partition blocks
            nc.vector.tensor_sub(pbt[:], pbt[:], nbt[:])
            btmp = consts.tile([128, 1], f32, tag="btmp")
            nc.vector.scalar_tensor_tensor(
                out=btmp[:], in0=pbt[:, 0:1], scalar=8.0, in1=pbt[:, 3:4],
                op0=mult, op1=add,
            )
            nc.vector.scalar_tensor_tensor(
                out=bias_vec[:], in0=pbt[:, 1:2], scalar=4.0, in1=btmp[:],
                op0=mult, op1=add,
            )
            nc.vector.scalar_tensor_tensor(
                out=btmp[:], in0=pbt[:, 2:3], scalar=2.0, in1=bias_vec[:],
                op0=mult, op1=add,
            )
            nc.vector.tensor_mul(btmp[:], btmp[:], bsv[:])
            nc.scalar.mul(bias_vec[:], btmp[:], 1.0 / 15.0)
            nc.scalar.mul(scale_vec[:], scale_vec[:], 1.0 / 15.0)

            # ---- main conv loop ----
            for b in range(BPC):
                xs = xs_next
                xs_next = xs_next2
                xs_next2 = load_image(b + 2) if b + 2 < BPC else None

                outb = opool.tile([128, GW], bf16, tag="outb")
                for n0, nt in N_TILES:
                    ps = pspool.tile([128, 512], f32, tag="ps")
                    for t, off in enumerate(TAP_OFFS):
                        nc.tensor.matmul(
                            ps[:, 0:nt],
                            lhsT[t][:],
                            xs[:, XBASE + n0 + off : XBASE + n0 + off + nt],
                            start=(t == 0),
                            stop=(t == 8),
                        )
                    nc.scalar.activation(
                        outb[:, n0 : n0 + nt],
                        ps[:, 0:nt],
                        mybir.ActivationFunctionType.Identity,
                        bias=bias_vec[:],
                        scale=scale_vec[:],
                    )

                # ---- edge-column repair (wrap contamination at c=0 and c=111) ----
                xv0 = xs[0:C, XBASE + W : XBASE + 58 * W].rearrange(
                    "p (r w) -> p r w", w=W
                )
                xv1 = xs[C:128, XBASE : XBASE + 57 * W].rearrange(
                    "p (r w) -> p r w", w=W
                )
                eR = epool.tile([128, 58], bf16, tag="eR")
                eL = epool.tile([128, 58], bf16, tag="eL")
                nc.gpsimd.memset(eR[:], 0)
                nc.gpsimd.memset(eL[:], 0)
                nc.vector.tensor_scalar_mul(
                    eR[0:C, 2:58], xv0[:, 0:56, W - 1], scale_vec[0:C]
                )
                nc.vector.tensor_scalar_mul(
                    eR[C:128, 1:58], xv1[:, 0:57, W - 1], scale_vec[C:128]
                )
                nc.vector.tensor_scalar_mul(
                    eL[0:C, 0:57], xv0[:, 0:57, 0], scale_vec[0:C]
                )
                nc.vector.tensor_scalar_mul(
                    eL[C:128, 0:56], xv1[:, 1:57, 0], scale_vec[C:128]
                )

                psF = fixpool.tile([128, 512], f32, tag="psF")
                ov = outb[:].rearrange("p (r w) -> p r w", w=W)
                # single fix phase normally; 2 phases + finer output chunks on the
                # last image so its writeback drains sooner (shorter tail)
                last = b == BPC - 1
                phases = [(0, 28), (28, 28)] if last else [(0, 56)]
                for qs, qn in phases:
                    for dr in range(3):
                        nc.tensor.matmul(
                            psF[:, qs : qs + qn],
                            lhsT[dr * 3 + 0][:],
                            eR[:, dr + qs : dr + qs + qn],
                            start=(dr == 0),
                            stop=(dr == 2),
                        )
                    for dr in range(3):
                        nc.tensor.matmul(
                            psF[:, 64 + qs : 64 + qs + qn],
                            lhsT[dr * 3 + 2][:],
                            eL[:, dr + qs : dr + qs + qn],
                            start=(dr == 0),
                            stop=(dr == 2),
                        )
                    nc.vector.tensor_sub(
                        ov[:, qs : qs + qn, 0],
                        ov[:, qs : qs + qn, 0],
                        psF[:, qs : qs + qn],
                    )
                    nc.vector.tensor_sub(
                        ov[:, qs : qs + qn, W - 1],
                        ov[:, qs : qs + qn, W - 1],
                        psF[:, 64 + qs : 64 + qs + qn],
                    )
                    # output DMA (bf16): top group on the sync HWDGE ring,
                    # bottom group on the scalar HWDGE ring (parallel issue)
                    nc.sync.dma_start(
                        y_d[b, :, qs : qs + qn, :],
                        outb[0:C, qs * W : (qs + qn) * W],
                    )
                    nc.scalar.dma_start(
                        y_d[b, :, HALF + qs : HALF + qs + qn, :],
                        outb[C:128, qs * W : (qs + qn) * W],
                    )

    nc.compile()
    _CACHE["nc"] = nc
    return nc


def _run(inputs, trace=False):
    from concourse.bass_utils import run_bass_kernel_spmd

    nc = _build()
    x = np.ascontiguousarray(np.asarray(inputs["x"], dtype=np.float32))
    shared = {
        "pweight": np.ascontiguousarray(np.asarray(inputs["pweight"], np.float32)),
        "nweight": np.ascontiguousarray(np.asarray(inputs["nweight"], np.float32)),
        "scale": np.ascontiguousarray(np.asarray(inputs["scale"], np.float32)),
        "pbias": np.ascontiguousarray(np.asarray(inputs["pbias"], np.float32)),
        "nbias": np.ascontiguousarray(np.asarray(inputs["nbias"], np.float32)),
        "biasscale": np.ascontiguousarray(np.asarray(inputs["biasscale"], np.float32)),
    }
    in_maps = [dict(shared, x=x[c * BPC : (c + 1) * BPC]) for c in range(CORES)]
    last_err = None
    for attempt in range(3):
        try:
            res = run_bass_kernel_spmd(
                nc, in_maps, core_ids=list(range(CORES)), trace=trace
            )
            out = np.concatenate(
                [np.asarray(res.results[c]["y"]) for c in range(CORES)], axis=0
            ).astype(np.float32)
            return out, res.exec_time_ns
        except Exception as e:  # transient NRT_EXEC_UNIT_UNRECOVERABLE recovers on retry
            last_err = e
            import time

            time.sleep(10)
    raise last_err


def kernel(**inputs) -> np.ndarray:
    out, _ = _run(inputs)
    return out


# revision 23
# speedup vs baseline: 1.0405x; 1.0405x over previous
"""BitConv2d forward on 8 Trainium2 NeuronCores (SPMD data-parallel).

Strategy:
  - Shard batch (32) -> 4 images per core; replicate the tiny bit-plane
    weights/scales on every core. No collectives needed (forward only).
  - On each core, reconstruct the integer weight planes on device:
        W_int[o,i,kh,kw] = sum_b (pweight-nweight)[...,b] * 2^(3-b)   (exact, in [-15,15])
    and fold scale/15 plus the bias into the PSUM->SBUF epilogue.
  - Everything on the x-side runs in bf16: the input is cast f32->bf16 during
    the SWDGE DMA (weights are small integers, exact in bf16; x keeps 8 mantissa
    bits -> ~1e-3 output error, far inside the 2e-2 gate). The output is written
    back as bf16 too, halving HBM write traffic (the DMA engines, ~205 GB/s
    aggregate, are the binding resource).
  - 3x3 same-pad conv as 9 accumulating matmuls per output tile. The image is
    row-flattened WITHOUT horizontal padding (rows contiguous at stride 112) so
    every transfer is a large contiguous block. Partitions 0:64 hold x rows
    -1..56 (top half), partitions 64:128 hold rows 55..112 (bottom half);
    block-diagonal [[W_t,0],[0,W_t]] stationaries serve both halves at once.
    Input DMAs are chunked (4 per half) so matmuls chase the incoming stream
    instead of waiting for whole images.
  - Without column padding, taps at dc=+-1 wrap across row boundaries and
    contaminate output columns 0 and 111; repaired by 6 tiny matmuls per image
    (3-tap 1-D convs over the edge columns, reusing the block-diag stationaries)
    plus a strided PSUM subtract on the vector engine.
"""

import numpy as np

B, C, H, W = 32, 64, 112, 112
NB = 4
CORES = 8
BPC = B // CORES  # images per core

HALF = H // 2            # 56 output rows per partition-group
GW = HALF * W            # 6272 output columns per group
XCOLS = 1 + W + 57 * W + 1  # guard + extra row + 57 data rows + guard = 6498
XBASE = 1                # column of x-row -1 (top) / x-row 55 (bottom)

N_TILES = [(i * 512, 512) for i in range(11)] + [(5632, 320), (5952, 320)]
TAP_OFFS = [kh * W + kw - 1 for kh in range(3) for kw in range(3)]
IN_CHUNKS = [(0, 15), (15, 14), (29, 14), (43, 14)]  # (row0, nrows) covering 57

_CACHE = {}


def _build():
    if "nc" in _CACHE:
        return _CACHE["nc"]
    import concourse.bacc as bacc
    import concourse.mybir as mybir
    from concourse import tile
    from concourse.masks import make_identity

    f32 = mybir.dt.float32
    bf16 = mybir.dt.bfloat16
    u32 = mybir.dt.uint32
    mult = mybir.AluOpType.mult
    add = mybir.AluOpType.add

    nc = bacc.Bacc("TRN2", target_bir_lowering=False, debug=False, num_devices=CORES)

    x_d = nc.dram_tensor("x", [BPC, C, H, W], f32, kind="ExternalInput").ap()
    pw_d = nc.dram_tensor("pweight", [C, C, 3, 3, NB], f32, kind="ExternalInput").ap()
    nw_d = nc.dram_tensor("nweight", [C, C, 3, 3, NB], f32, kind="ExternalInput").ap()
    sc_d = nc.dram_tensor("scale", [1], f32, kind="ExternalInput").ap()
    pb_d = nc.dram_tensor("pbias", [C, NB], f32, kind="ExternalInput").ap()
    nb_d = nc.dram_tensor("nbias", [C, NB], f32, kind="ExternalInput").ap()
    bs_d = nc.dram_tensor("biasscale", [1], f32, kind="ExternalInput").ap()
    y_d = nc.dram_tensor("y", [BPC, C, H, W], bf16, kind="ExternalOutput").ap()

    with tile.TileContext(nc) as tc:
        with (
            tc.tile_pool(name="consts", bufs=1) as consts,
            tc.tile_pool(name="xpool", bufs=2) as xpool,
            tc.tile_pool(name="opool", bufs=2) as opool,
            tc.tile_pool(name="epool", bufs=2) as epool,
            tc.tile_pool(name="pspool", bufs=4, space="PSUM") as pspool,
            tc.tile_pool(name="fixpool", bufs=1, space="PSUM") as fixpool,
            tc.tile_pool(name="psum_t", bufs=1, space="PSUM") as psum_t,
        ):
            ident = consts.tile([128, 128], bf16, tag="ident")
            lhsT = [
                consts.tile([128, 128], bf16, tag=f"lhsT{t}", name=f"lhsT{t}")
                for t in range(9)
            ]
            scale_vec = consts.tile([128, 1], f32, tag="scale_vec")
            bias_vec = consts.tile([128, 1], f32, tag="bias_vec")

            # the identity (for weight transposes) must be FIRST on the Q7 queue:
            # everything queued behind the image-chunk descriptor generation
            # arrives ~10us late otherwise
            make_identity(nc, ident[:])

            # ---- weight DMAs: SWDGE bf16 cast, queued on Q7 right after the
            # identity and ahead of the image chunks (~0.6 MB, lands ~10us) ----
            # [128, 1152] layout: partition (h,o) holds W[o, i=32h..32h+31, :, :, :]
            wp = consts.tile([128, 32 * 9 * NB], bf16, tag="wp")
            wn = consts.tile([128, 32 * 9 * NB], bf16, tag="wn")
            pw_f = pw_d.rearrange("o i kh kw b -> o (i kh kw b)")
            nw_f = nw_d.rearrange("o i kh kw b -> o (i kh kw b)")
            nc.gpsimd.dma_start(wp[0:C, :], pw_f[:, 0:1152])
            nc.gpsimd.dma_start(wp[C:128, :], pw_f[:, 1152:2304])
            nc.gpsimd.dma_start(wn[0:C, :], nw_f[:, 0:1152])
            nc.gpsimd.dma_start(wn[C:128, :], nw_f[:, 1152:2304])
            pbt = consts.tile([128, NB], f32, tag="pbt")
            nbt = consts.tile([128, NB], f32, tag="nbt")
            nc.sync.dma_start(pbt[0:C, :], pb_d)
            nc.sync.dma_start(pbt[C:128, :], pb_d)
            nc.sync.dma_start(nbt[0:C, :], nb_d)
            nc.sync.dma_start(nbt[C:128, :], nb_d)
            bsv = consts.tile([128, 1], f32, tag="bsv")
            nc.sync.dma_start(bsv[:], bs_d.to_broadcast((128, 1)))
            nc.sync.dma_start(scale_vec[:], sc_d.to_broadcast((128, 1)))

            # ---- image load pipeline: chunked SWDGE cast-DMAs (f32 -> bf16) ----
            # fine chunks for image 0 (compute chases the stream); coarse after
            # (Q7 descriptor-gen is ~1us per call). DMAs before memsets: the
            # ranges are disjoint, and the DMAs are the critical path.
            def load_image(b):
                xs = xpool.tile([128, XCOLS], bf16, tag="xs", name=f"xs{b}", bufs=3)
                v0 = xs[0:C, XBASE + W : XBASE + W + 57 * W].rearrange(
                    "p (r w) -> p r w", w=W
                )
                v1 = xs[C:128, XBASE : XBASE + 57 * W].rearrange(
                    "p (r w) -> p r w", w=W
                )
                chunks = IN_CHUNKS if b == 0 else [(0, 29), (29, 28)]
                for r0, nr in chunks:
                    nc.gpsimd.dma_start(v0[:, r0 : r0 + nr, :], x_d[b, :, r0 : r0 + nr, :])
                    nc.gpsimd.dma_start(
                        v1[:, r0 : r0 + nr, :], x_d[b, :, 55 + r0 : 55 + r0 + nr, :]
                    )
                nc.gpsimd.memset(xs[0:C, 0 : XBASE + W], 0)
                nc.gpsimd.memset(xs[0:C, XCOLS - 1 : XCOLS], 0)
                nc.gpsimd.memset(xs[C:128, 0:XBASE], 0)
                nc.gpsimd.memset(xs[C:128, XBASE + 57 * W : XCOLS], 0)
                return xs

            xs_next = load_image(0)
            xs_next2 = load_image(1)

            # ---- weight/bias reconstruction (overlaps image-0 DMA) ----
            # Processed in 3 transpose-groups (taps 0-3 / 4-7 / 8) so lhsT[0]
            # is ready early and the first conv matmuls start while the rest
            # of the weight prep continues.
            nc.vector.tensor_sub(wp[:], wp[:], wn[:])  # d = p - n
            # bit-combine into tap-major W_int [(h,o), (t, il)]:
            # w = ((d0*8 + d3) + d1*4) + d2*2 via scalar_tensor_tensor chains
            # (f32 in, bf16 out on the last write; integers <=15 stay exact)
            wi = consts.tile([128, 9 * 32], bf16, tag="wi")
            wt2 = consts.tile([128, 9 * 32], bf16, tag="wt2")
            wi_v = wi[:].rearrange("p (t i) -> p t i", t=9)
            wt2_v = wt2[:].rearrange("p (t i) -> p t i", t=9)
            d_v = wp[:].rearrange("p (i t b) -> p t i b", t=9, b=NB)
            for t in range(9):
                nc.vector.memset(lhsT[t][:], 0)
            groups = [(0, 4), (4, 4), (8, 1)]
            tps_l = []
            for g, (tp0, ntp) in enumerate(groups):
                dv = d_v[:, tp0 : tp0 + ntp]
                wiv = wi_v[:, tp0 : tp0 + ntp]
                wtv = wt2_v[:, tp0 : tp0 + ntp]
                nc.vector.scalar_tensor_tensor(
                    out=wtv, in0=dv[:, :, :, 0], scalar=8.0, in1=dv[:, :, :, 3],
                    op0=mult, op1=add,
                )
                nc.vector.scalar_tensor_tensor(
                    out=wiv, in0=dv[:, :, :, 1], scalar=4.0, in1=wtv,
                    op0=mult, op1=add,
                )
                nc.vector.scalar_tensor_tensor(
                    out=wtv, in0=dv[:, :, :, 2], scalar=2.0, in1=wiv,
                    op0=mult, op1=add,
                )
                cols = 32 * ntp
                tps = psum_t.tile([128, 128], bf16, tag="tps", name=f"tps{g}", bufs=3)
                nc.tensor.transpose(
                    tps[0:cols, :], wt2[:, tp0 * 32 : tp0 * 32 + cols], ident[:]
                )
                tps_l.append(tps)
            for g, (tp0, ntp) in enumerate(groups):
                tps = tps_l[g]
                for t in range(tp0, tp0 + ntp):
                    rt = (t - tp0) * 32
                    nc.scalar.copy(lhsT[t][0:32, 0:C], tps[rt : rt + 32, 0:C])
                    nc.vector.tensor_copy(
                        lhsT[t][32:C, 0:C], tps[rt : rt + 32, C:128]
                    )
                    nc.vector.tensor_copy(
                        lhsT[t][C : C + 32, C:128], tps[rt : rt + 32, 0:C]
                    )
                    nc.vector.tensor_copy(
                        lhsT[t][C + 32 : 128, C:128], tps[rt : rt + 32, C:128]
                    )
            # bias vector, duplicated across both partition blocks
            nc.vector.tensor_sub(pbt[:], pbt[:], nbt[:])
            btmp = consts.tile([128, 1], f32, tag="btmp")
            nc.vector.scalar_tensor_tensor(
                out=btmp[:], in0=pbt[:, 0:1], scalar=8.0, in1=pbt[:, 3:4],
                op0=mult, op1=add,
            )
            nc.vector.scalar_tensor_tensor(
                out=bias_vec[:], in0=pbt[:, 1:2], scalar=4.0, in1=btmp[:],
                op0=mult, op1=add,
            )
            nc.vector.scalar_tensor_tensor(
                out=btmp[:], in0=pbt[:, 2:3], scalar=2.0, in1=bias_vec[:],
                op0=mult, op1=add,
            )
            nc.vector.tensor_mul(btmp[:], btmp[:], bsv[:])
            nc.scalar.mul(bias_vec[:], btmp[:], 1.0 / 15.0)
            nc.scalar.mul(scale_vec[:], scale_vec[:], 1.0 / 15.0)

            # ---- main conv loop ----
            for b in range(BPC):
                xs = xs_next
                xs_next = xs_next2
                xs_next2 = load_image(b + 2) if b + 2 < BPC else None

                outb = opool.tile([128, GW], bf16, tag="outb")
                ov = outb[:].rearrange("p (r w) -> p r w", w=W)
                last = b == BPC - 1

                def emit_tiles(tiles, xs=xs, outb=outb):
                    for n0, nt in tiles:
                        ps = pspool.tile([128, 512], f32, tag="ps")
                        for t, off in enumerate(TAP_OFFS):
                            nc.tensor.matmul(
                                ps[:, 0:nt],
                                lhsT[t][:],
                                xs[:, XBASE + n0 + off : XBASE + n0 + off + nt],
                                start=(t == 0),
                                stop=(t == 8),
                            )
                        nc.scalar.activation(
                            outb[:, n0 : n0 + nt],
                            ps[:, 0:nt],
                            mybir.ActivationFunctionType.Identity,
                            bias=bias_vec[:],
                            scale=scale_vec[:],
                        )

                def emit_gathers(xs=xs):
                    # edge sequences (wrap contamination sources), pre-scaled
                    xv0 = xs[0:C, XBASE + W : XBASE + 58 * W].rearrange(
                        "p (r w) -> p r w", w=W
                    )
                    xv1 = xs[C:128, XBASE : XBASE + 57 * W].rearrange(
                        "p (r w) -> p r w", w=W
                    )
                    eR = epool.tile([128, 58], bf16, tag="eR")
                    eL = epool.tile([128, 58], bf16, tag="eL")
                    nc.vector.memset(eR[:], 0)
                    nc.vector.memset(eL[:], 0)
                    nc.vector.tensor_scalar_mul(
                        eR[0:C, 2:58], xv0[:, 0:56, W - 1], scale_vec[0:C]
                    )
                    nc.vector.tensor_scalar_mul(
                        eR[C:128, 1:58], xv1[:, 0:57, W - 1], scale_vec[C:128]
                    )
                    nc.vector.tensor_scalar_mul(
                        eL[0:C, 0:57], xv0[:, 0:57, 0], scale_vec[0:C]
                    )
                    nc.vector.tensor_scalar_mul(
                        eL[C:128, 0:56], xv1[:, 1:57, 0], scale_vec[C:128]
                    )
                    return eR, eL

                def emit_fix_out(psF, eR, eL, qs, qn, b=b, ov=ov, outb=outb):
                    # repair output columns 0 and 111, then stream the rows out
                    for dr in range(3):
                        nc.tensor.matmul(
                            psF[:, qs : qs + qn],
                            lhsT[dr * 3 + 0][:],
                            eR[:, dr + qs : dr + qs + qn],
                            start=(dr == 0),
                            stop=(dr == 2),
                        )
                    for dr in range(3):
                        nc.tensor.matmul(
                            psF[:, 64 + qs : 64 + qs + qn],
                            lhsT[dr * 3 + 2][:],
                            eL[:, dr + qs : dr + qs + qn],
                            start=(dr == 0),
                            stop=(dr == 2),
                        )
                    nc.vector.tensor_sub(
                        ov[:, qs : qs + qn, 0],
                        ov[:, qs : qs + qn, 0],
                        psF[:, qs : qs + qn],
                    )
                    nc.vector.tensor_sub(
                        ov[:, qs : qs + qn, W - 1],
                        ov[:, qs : qs + qn, W - 1],
                        psF[:, 64 + qs : 64 + qs + qn],
                    )
                    # top group on the sync HWDGE ring, bottom on the scalar ring
                    nc.sync.dma_start(
                        y_d[b, :, qs : qs + qn, :],
                        outb[0:C, qs * W : (qs + qn) * W],
                    )
                    nc.scalar.dma_start(
                        y_d[b, :, HALF + qs : HALF + qs + qn, :],
                        outb[C:128, qs * W : (qs + qn) * W],
                    )

                psF = fixpool.tile([128, 512], f32, tag="psF")
                if last:
                    # interleave: rows 0..27 are fixed and written back while
                    # tiles 7..12 still compute -> only rows 28..55 in the tail
                    emit_tiles(N_TILES[:7])
                    eR, eL = emit_gathers()
                    emit_fix_out(psF, eR, eL, 0, 28)
                    emit_tiles(N_TILES[7:])
                    emit_fix_out(psF, eR, eL, 28, 28)
                else:
                    emit_tiles(N_TILES)
                    eR, eL = emit_gathers()
                    emit_fix_out(psF, eR, eL, 0, 56)

    nc.compile()
    _CACHE["nc"] = nc
    return nc


def _run(inputs, trace=False):
    from concourse.bass_utils import run_bass_kernel_spmd

    nc = _build()
    x = np.ascontiguousarray(np.asarray(inputs["x"], dtype=np.float32))
    shared = {
        "pweight": np.ascontiguousarray(np.asarray(inputs["pweight"], np.float32)),
        "nweight": np.ascontiguousarray(np.asarray(inputs["nweight"], np.float32)),
        "scale": np.ascontiguousarray(np.asarray(inputs["scale"], np.float32)),
        "pbias": np.ascontiguousarray(np.asarray(inputs["pbias"], np.float32)),
        "nbias": np.ascontiguousarray(np.asarray(inputs["nbias"], np.float32)),
        "biasscale": np.ascontiguousarray(np.asarray(inputs["biasscale"], np.float32)),
    }
    in_maps = [dict(shared, x=x[c * BPC : (c + 1) * BPC]) for c in range(CORES)]
    last_err = None
    for attempt in range(3):
        try:
            res = run_bass_kernel_spmd(
                nc, in_maps, core_ids=list(range(CORES)), trace=trace
            )
            out = np.concatenate(
                [np.asarray(res.results[c]["y"]) for c in range(CORES)], axis=0
            ).astype(np.float32)
            return out, res.exec_time_ns
        except Exception as e:  # transient NRT_EXEC_UNIT_UNRECOVERABLE recovers on retry
            last_err = e
            import time

            time.sleep(10)
    raise last_err


def kernel(**inputs) -> np.ndarray:
    out, _ = _run(inputs)
    return out


# revision 26
# speedup vs baseline: 1.0708x; 1.0292x over previous
"""BitConv2d forward on 8 Trainium2 NeuronCores (SPMD data-parallel).

Strategy:
  - Shard batch (32) -> 4 images per core; replicate the tiny bit-plane
    weights/scales on every core. No collectives needed (forward only).
  - On each core, reconstruct the integer weight planes on device:
        W_int[o,i,kh,kw] = sum_b (pweight-nweight)[...,b] * 2^(3-b)   (exact, in [-15,15])
    and fold scale/15 plus the bias into the PSUM->SBUF epilogue.
  - Everything on the x-side runs in bf16: the input is cast f32->bf16 during
    the SWDGE DMA (weights are small integers, exact in bf16; x keeps 8 mantissa
    bits -> ~1e-3 output error, far inside the 2e-2 gate). The output is written
    back as bf16 too, halving HBM write traffic (the DMA engines, ~205 GB/s
    aggregate, are the binding resource).
  - 3x3 same-pad conv as 9 accumulating matmuls per output tile. The image is
    row-flattened WITHOUT horizontal padding (rows contiguous at stride 112) so
    every transfer is a large contiguous block. Partitions 0:64 hold x rows
    -1..56 (top half), partitions 64:128 hold rows 55..112 (bottom half);
    block-diagonal [[W_t,0],[0,W_t]] stationaries serve both halves at once.
    Input DMAs are chunked (4 per half) so matmuls chase the incoming stream
    instead of waiting for whole images.
  - Without column padding, taps at dc=+-1 wrap across row boundaries and
    contaminate output columns 0 and 111; repaired by 6 tiny matmuls per image
    (3-tap 1-D convs over the edge columns, reusing the block-diag stationaries)
    plus a strided PSUM subtract on the vector engine.
"""

import numpy as np

B, C, H, W = 32, 64, 112, 112
NB = 4
CORES = 8
BPC = B // CORES  # images per core

HALF = H // 2            # 56 output rows per partition-group
GW = HALF * W            # 6272 output columns per group
XCOLS = 1 + W + 57 * W + 1  # guard + extra row + 57 data rows + guard = 6498
XBASE = 1                # column of x-row -1 (top) / x-row 55 (bottom)

N_TILES = [(i * 512, 512) for i in range(11)] + [(5632, 320), (5952, 320)]
TAP_OFFS = [kh * W + kw - 1 for kh in range(3) for kw in range(3)]
IN_CHUNKS = [(0, 15), (15, 14), (29, 14), (43, 14)]  # (row0, nrows) covering 57

_CACHE = {}


def _build():
    if "nc" in _CACHE:
        return _CACHE["nc"]
    import concourse.bacc as bacc
    import concourse.mybir as mybir
    from concourse import tile
    from concourse.masks import make_identity

    f32 = mybir.dt.float32
    bf16 = mybir.dt.bfloat16
    u32 = mybir.dt.uint32
    mult = mybir.AluOpType.mult
    add = mybir.AluOpType.add

    nc = bacc.Bacc("TRN2", target_bir_lowering=False, debug=False, num_devices=CORES)

    x_d = nc.dram_tensor("x", [BPC, C, H, W], f32, kind="ExternalInput").ap()
    pw_d = nc.dram_tensor("pweight", [C, C, 3, 3, NB], f32, kind="ExternalInput").ap()
    nw_d = nc.dram_tensor("nweight", [C, C, 3, 3, NB], f32, kind="ExternalInput").ap()
    sc_d = nc.dram_tensor("scale", [1], f32, kind="ExternalInput").ap()
    pb_d = nc.dram_tensor("pbias", [C, NB], f32, kind="ExternalInput").ap()
    nb_d = nc.dram_tensor("nbias", [C, NB], f32, kind="ExternalInput").ap()
    bs_d = nc.dram_tensor("biasscale", [1], f32, kind="ExternalInput").ap()
    y_d = nc.dram_tensor("y", [BPC, C, H, W], bf16, kind="ExternalOutput").ap()

    with tile.TileContext(nc) as tc:
        with (
            tc.tile_pool(name="consts", bufs=1) as consts,
            tc.tile_pool(name="xpool", bufs=2) as xpool,
            tc.tile_pool(name="opool", bufs=2) as opool,
            tc.tile_pool(name="epool", bufs=4) as epool,
            tc.tile_pool(name="pspool", bufs=4, space="PSUM") as pspool,
            tc.tile_pool(name="fixpool", bufs=1, space="PSUM") as fixpool,
            tc.tile_pool(name="psum_t", bufs=1, space="PSUM") as psum_t,
        ):
            ident = consts.tile([128, 128], bf16, tag="ident")
            lhsT = [
                consts.tile([128, 128], bf16, tag=f"lhsT{t}", name=f"lhsT{t}")
                for t in range(9)
            ]
            scale_vec = consts.tile([128, 1], f32, tag="scale_vec")
            bias_vec = consts.tile([128, 1], f32, tag="bias_vec")

            # the identity (for weight transposes) must be FIRST on the Q7 queue:
            # everything queued behind the image-chunk descriptor generation
            # arrives ~10us late otherwise
            make_identity(nc, ident[:])

            # ---- weight DMAs: SWDGE bf16 cast, queued on Q7 right after the
            # identity and ahead of the image chunks (~0.6 MB, lands ~10us) ----
            # [128, 1152] layout: partition (h,o) holds W[o, i=32h..32h+31, :, :, :]
            wp = consts.tile([128, 32 * 9 * NB], bf16, tag="wp")
            wn = consts.tile([128, 32 * 9 * NB], bf16, tag="wn")
            pw_f = pw_d.rearrange("o i kh kw b -> o (i kh kw b)")
            nw_f = nw_d.rearrange("o i kh kw b -> o (i kh kw b)")
            nc.gpsimd.dma_start(wp[0:C, :], pw_f[:, 0:1152])
            nc.gpsimd.dma_start(wp[C:128, :], pw_f[:, 1152:2304])
            nc.gpsimd.dma_start(wn[0:C, :], nw_f[:, 0:1152])
            nc.gpsimd.dma_start(wn[C:128, :], nw_f[:, 1152:2304])
            pbt = consts.tile([128, NB], f32, tag="pbt")
            nbt = consts.tile([128, NB], f32, tag="nbt")
            nc.sync.dma_start(pbt[0:C, :], pb_d)
            nc.sync.dma_start(pbt[C:128, :], pb_d)
            nc.sync.dma_start(nbt[0:C, :], nb_d)
            nc.sync.dma_start(nbt[C:128, :], nb_d)
            bsv = consts.tile([128, 1], f32, tag="bsv")
            nc.sync.dma_start(bsv[:], bs_d.to_broadcast((128, 1)))
            nc.sync.dma_start(scale_vec[:], sc_d.to_broadcast((128, 1)))

            # ---- image load pipeline: chunked SWDGE cast-DMAs (f32 -> bf16) ----
            # fine chunks for image 0 (compute chases the stream); coarse after
            # (Q7 descriptor-gen is ~1us per call). DMAs before memsets: the
            # ranges are disjoint, and the DMAs are the critical path.
            def load_image(b):
                xs = xpool.tile([128, XCOLS], bf16, tag="xs", name=f"xs{b}", bufs=3)
                v0 = xs[0:C, XBASE + W : XBASE + W + 57 * W].rearrange(
                    "p (r w) -> p r w", w=W
                )
                v1 = xs[C:128, XBASE : XBASE + 57 * W].rearrange(
                    "p (r w) -> p r w", w=W
                )
                # memsets FIRST: on the in-order Q7 queue they carry the
                # buffer-reuse (WAR) gate for the chunk DMAs behind them
                nc.gpsimd.memset(xs[0:C, 0 : XBASE + W], 0)
                nc.gpsimd.memset(xs[0:C, XCOLS - 1 : XCOLS], 0)
                nc.gpsimd.memset(xs[C:128, 0:XBASE], 0)
                nc.gpsimd.memset(xs[C:128, XBASE + 57 * W : XCOLS], 0)
                chunks = IN_CHUNKS if b == 0 else [(0, 29), (29, 28)]
                for r0, nr in chunks:
                    nc.gpsimd.dma_start(v0[:, r0 : r0 + nr, :], x_d[b, :, r0 : r0 + nr, :])
                    nc.gpsimd.dma_start(
                        v1[:, r0 : r0 + nr, :], x_d[b, :, 55 + r0 : 55 + r0 + nr, :]
                    )
                return xs

            xs_next = load_image(0)
            xs_next2 = load_image(1)

            # ---- weight/bias reconstruction (overlaps image-0 DMA) ----
            # Processed in 3 transpose-groups (taps 0-3 / 4-7 / 8) so lhsT[0]
            # is ready early and the first conv matmuls start while the rest
            # of the weight prep continues.
            nc.vector.tensor_sub(wp[:], wp[:], wn[:])  # d = p - n
            # bit-combine into tap-major W_int [(h,o), (t, il)]:
            # w = ((d0*8 + d3) + d1*4) + d2*2 via scalar_tensor_tensor chains
            # (f32 in, bf16 out on the last write; integers <=15 stay exact)
            wi = consts.tile([128, 9 * 32], bf16, tag="wi")
            wt2 = consts.tile([128, 9 * 32], bf16, tag="wt2")
            wi_v = wi[:].rearrange("p (t i) -> p t i", t=9)
            wt2_v = wt2[:].rearrange("p (t i) -> p t i", t=9)
            d_v = wp[:].rearrange("p (i t b) -> p t i b", t=9, b=NB)
            for t in range(9):
                nc.vector.memset(lhsT[t][:], 0)
            groups = [(0, 4), (4, 4), (8, 1)]
            tps_l = []
            for g, (tp0, ntp) in enumerate(groups):
                dv = d_v[:, tp0 : tp0 + ntp]
                wiv = wi_v[:, tp0 : tp0 + ntp]
                wtv = wt2_v[:, tp0 : tp0 + ntp]
                nc.vector.scalar_tensor_tensor(
                    out=wtv, in0=dv[:, :, :, 0], scalar=8.0, in1=dv[:, :, :, 3],
                    op0=mult, op1=add,
                )
                nc.vector.scalar_tensor_tensor(
                    out=wiv, in0=dv[:, :, :, 1], scalar=4.0, in1=wtv,
                    op0=mult, op1=add,
                )
                nc.vector.scalar_tensor_tensor(
                    out=wtv, in0=dv[:, :, :, 2], scalar=2.0, in1=wiv,
                    op0=mult, op1=add,
                )
                cols = 32 * ntp
                tps = psum_t.tile([128, 128], bf16, tag="tps", name=f"tps{g}", bufs=3)
                nc.tensor.transpose(
                    tps[0:cols, :], wt2[:, tp0 * 32 : tp0 * 32 + cols], ident[:]
                )
                tps_l.append(tps)
            for g, (tp0, ntp) in enumerate(groups):
                tps = tps_l[g]
                for t in range(tp0, tp0 + ntp):
                    rt = (t - tp0) * 32
                    nc.scalar.copy(lhsT[t][0:32, 0:C], tps[rt : rt + 32, 0:C])
                    nc.vector.tensor_copy(
                        lhsT[t][32:C, 0:C], tps[rt : rt + 32, C:128]
                    )
                    nc.vector.tensor_copy(
                        lhsT[t][C : C + 32, C:128], tps[rt : rt + 32, 0:C]
                    )
                    nc.vector.tensor_copy(
                        lhsT[t][C + 32 : 128, C:128], tps[rt : rt + 32, C:128]
                    )
            # bias vector, duplicated across both partition blocks
            nc.vector.tensor_sub(pbt[:], pbt[:], nbt[:])
            btmp = consts.tile([128, 1], f32, tag="btmp")
            nc.vector.scalar_tensor_tensor(
                out=btmp[:], in0=pbt[:, 0:1], scalar=8.0, in1=pbt[:, 3:4],
                op0=mult, op1=add,
            )
            nc.vector.scalar_tensor_tensor(
                out=bias_vec[:], in0=pbt[:, 1:2], scalar=4.0, in1=btmp[:],
                op0=mult, op1=add,
            )
            nc.vector.scalar_tensor_tensor(
                out=btmp[:], in0=pbt[:, 2:3], scalar=2.0, in1=bias_vec[:],
                op0=mult, op1=add,
            )
            nc.vector.tensor_mul(btmp[:], btmp[:], bsv[:])
            nc.scalar.mul(bias_vec[:], btmp[:], 1.0 / 15.0)
            nc.scalar.mul(scale_vec[:], scale_vec[:], 1.0 / 15.0)

            # ---- main conv loop ----
            for b in range(BPC):
                xs = xs_next
                xs_next = xs_next2
                xs_next2 = load_image(b + 2) if b + 2 < BPC else None

                outb = opool.tile([128, GW], bf16, tag="outb")
                ov = outb[:].rearrange("p (r w) -> p r w", w=W)
                last = b == BPC - 1

                def emit_tiles(tiles, xs=xs, outb=outb):
                    for n0, nt in tiles:
                        ps = pspool.tile([128, 512], f32, tag="ps")
                        for t, off in enumerate(TAP_OFFS):
                            nc.tensor.matmul(
                                ps[:, 0:nt],
                                lhsT[t][:],
                                xs[:, XBASE + n0 + off : XBASE + n0 + off + nt],
                                start=(t == 0),
                                stop=(t == 8),
                            )
                        nc.scalar.activation(
                            outb[:, n0 : n0 + nt],
                            ps[:, 0:nt],
                            mybir.ActivationFunctionType.Identity,
                            bias=bias_vec[:],
                            scale=scale_vec[:],
                        )

                def emit_gathers(xs=xs):
                    # edge sequences (wrap contamination sources), pre-scaled
                    xv0 = xs[0:C, XBASE + W : XBASE + 58 * W].rearrange(
                        "p (r w) -> p r w", w=W
                    )
                    xv1 = xs[C:128, XBASE : XBASE + 57 * W].rearrange(
                        "p (r w) -> p r w", w=W
                    )
                    eR = epool.tile([128, 58], bf16, tag="eR", name=f"eR{b}")
                    eL = epool.tile([128, 58], bf16, tag="eL", name=f"eL{b}")
                    nc.gpsimd.memset(eR[:], 0)
                    nc.gpsimd.memset(eL[:], 0)
                    nc.vector.tensor_scalar_mul(
                        eR[0:C, 2:58], xv0[:, 0:56, W - 1], scale_vec[0:C]
                    )
                    nc.vector.tensor_scalar_mul(
                        eR[C:128, 1:58], xv1[:, 0:57, W - 1], scale_vec[C:128]
                    )
                    nc.vector.tensor_scalar_mul(
                        eL[0:C, 0:57], xv0[:, 0:57, 0], scale_vec[0:C]
                    )
                    nc.vector.tensor_scalar_mul(
                        eL[C:128, 0:56], xv1[:, 1:57, 0], scale_vec[C:128]
                    )
                    return eR, eL

                def emit_fix_out(psF, eR, eL, qs, qn, b=b, ov=ov, outb=outb):
                    # repair output columns 0 and 111, then stream the rows out
                    for dr in range(3):
                        nc.tensor.matmul(
                            psF[:, qs : qs + qn],
                            lhsT[dr * 3 + 0][:],
                            eR[:, dr + qs : dr + qs + qn],
                            start=(dr == 0),
                            stop=(dr == 2),
                        )
                    for dr in range(3):
                        nc.tensor.matmul(
                            psF[:, 64 + qs : 64 + qs + qn],
                            lhsT[dr * 3 + 2][:],
                            eL[:, dr + qs : dr + qs + qn],
                            start=(dr == 0),
                            stop=(dr == 2),
                        )
                    nc.vector.tensor_sub(
                        ov[:, qs : qs + qn, 0],
                        ov[:, qs : qs + qn, 0],
                        psF[:, qs : qs + qn],
                    )
                    nc.vector.tensor_sub(
                        ov[:, qs : qs + qn, W - 1],
                        ov[:, qs : qs + qn, W - 1],
                        psF[:, 64 + qs : 64 + qs + qn],
                    )
                    # top group on the sync HWDGE ring, bottom on the scalar ring
                    nc.sync.dma_start(
                        y_d[b, :, qs : qs + qn, :],
                        outb[0:C, qs * W : (qs + qn) * W],
                    )
                    nc.scalar.dma_start(
                        y_d[b, :, HALF + qs : HALF + qs + qn, :],
                        outb[C:128, qs * W : (qs + qn) * W],
                    )

                psF = fixpool.tile([128, 512], f32, tag="psF")
                if last:
                    # interleave: rows 0..27 are fixed and written back while
                    # tiles 7..12 still compute -> only rows 28..55 in the tail
                    emit_tiles(N_TILES[:7])
                    eR, eL = emit_gathers()
                    emit_fix_out(psF, eR, eL, 0, 28)
                    emit_tiles(N_TILES[7:])
                    emit_fix_out(psF, eR, eL, 28, 28)
                else:
                    emit_tiles(N_TILES)
                    eR, eL = emit_gathers()
                    emit_fix_out(psF, eR, eL, 0, 56)

    nc.compile()
    _CACHE["nc"] = nc
    return nc


def _run(inputs, trace=False):
    from concourse.bass_utils import run_bass_kernel_spmd

    nc = _build()
    x = np.ascontiguousarray(np.asarray(inputs["x"], dtype=np.float32))
    shared = {
        "pweight": np.ascontiguousarray(np.asarray(inputs["pweight"], np.float32)),
        "nweight": np.ascontiguousarray(np.asarray(inputs["nweight"], np.float32)),
        "scale": np.ascontiguousarray(np.asarray(inputs["scale"], np.float32)),
        "pbias": np.ascontiguousarray(np.asarray(inputs["pbias"], np.float32)),
        "nbias": np.ascontiguousarray(np.asarray(inputs["nbias"], np.float32)),
        "biasscale": np.ascontiguousarray(np.asarray(inputs["biasscale"], np.float32)),
    }
    in_maps = [dict(shared, x=x[c * BPC : (c + 1) * BPC]) for c in range(CORES)]
    last_err = None
    for attempt in range(3):
        try:
            res = run_bass_kernel_spmd(
                nc, in_maps, core_ids=list(range(CORES)), trace=trace
            )
            out = np.concatenate(
                [np.asarray(res.results[c]["y"]) for c in range(CORES)], axis=0
            ).astype(np.float32)
            return out, res.exec_time_ns
        except Exception as e:  # transient NRT_EXEC_UNIT_UNRECOVERABLE recovers on retry
            last_err = e
            import time

            time.sleep(10)
    raise last_err


def kernel(**inputs) -> np.ndarray:
    out, _ = _run(inputs)
    return out


# revision 31
# speedup vs baseline: 1.0733x; 1.0023x over previous
"""BitConv2d forward on 8 Trainium2 NeuronCores (SPMD data-parallel).

Strategy:
  - Shard batch (32) -> 4 images per core; replicate the tiny bit-plane
    weights/scales on every core. No collectives needed (forward only).
  - On each core, reconstruct the integer weight planes on device:
        W_int[o,i,kh,kw] = sum_b (pweight-nweight)[...,b] * 2^(3-b)   (exact, in [-15,15])
    and fold scale/15 plus the bias into the PSUM->SBUF epilogue.
  - Everything on the x-side runs in bf16: the input is cast f32->bf16 during
    the SWDGE DMA (weights are small integers, exact in bf16; x keeps 8 mantissa
    bits -> ~1e-3 output error, far inside the 2e-2 gate). The output is written
    back as bf16 too, halving HBM write traffic (the DMA engines, ~205 GB/s
    aggregate, are the binding resource).
  - 3x3 same-pad conv as 9 accumulating matmuls per output tile. The image is
    row-flattened WITHOUT horizontal padding (rows contiguous at stride 112) so
    every transfer is a large contiguous block. Partitions 0:64 hold x rows
    -1..56 (top half), partitions 64:128 hold rows 55..112 (bottom half);
    block-diagonal [[W_t,0],[0,W_t]] stationaries serve both halves at once.
    Input DMAs are chunked (4 per half) so matmuls chase the incoming stream
    instead of waiting for whole images.
  - Without column padding, taps at dc=+-1 wrap across row boundaries and
    contaminate output columns 0 and 111; repaired by 6 tiny matmuls per image
    (3-tap 1-D convs over the edge columns, reusing the block-diag stationaries)
    plus a strided PSUM subtract on the vector engine.
"""

import numpy as np

B, C, H, W = 32, 64, 112, 112
NB = 4
CORES = 8
BPC = B // CORES  # images per core

HALF = H // 2            # 56 output rows per partition-group
GW = HALF * W            # 6272 output columns per group
XCOLS = 1 + W + 57 * W + 1  # guard + extra row + 57 data rows + guard = 6498
XBASE = 1                # column of x-row -1 (top) / x-row 55 (bottom)

N_TILES = [(i * 512, 512) for i in range(11)] + [(5632, 320), (5952, 320)]
TAP_OFFS = [kh * W + kw - 1 for kh in range(3) for kw in range(3)]
IN_CHUNKS = [(0, 15), (15, 14), (29, 14), (43, 14)]  # (row0, nrows) covering 57

_CACHE = {}


def _build():
    if "nc" in _CACHE:
        return _CACHE["nc"]
    import concourse.bacc as bacc
    import concourse.mybir as mybir
    from concourse import tile
    from concourse.masks import make_identity

    f32 = mybir.dt.float32
    bf16 = mybir.dt.bfloat16
    u32 = mybir.dt.uint32
    mult = mybir.AluOpType.mult
    add = mybir.AluOpType.add

    nc = bacc.Bacc("TRN2", target_bir_lowering=False, debug=False, num_devices=CORES)

    x_d = nc.dram_tensor("x", [BPC, C, H, W], f32, kind="ExternalInput").ap()
    pw_d = nc.dram_tensor("pweight", [C, C, 3, 3, NB], f32, kind="ExternalInput").ap()
    nw_d = nc.dram_tensor("nweight", [C, C, 3, 3, NB], f32, kind="ExternalInput").ap()
    sc_d = nc.dram_tensor("scale", [1], f32, kind="ExternalInput").ap()
    pb_d = nc.dram_tensor("pbias", [C, NB], f32, kind="ExternalInput").ap()
    nb_d = nc.dram_tensor("nbias", [C, NB], f32, kind="ExternalInput").ap()
    bs_d = nc.dram_tensor("biasscale", [1], f32, kind="ExternalInput").ap()
    y_d = nc.dram_tensor("y", [BPC, C, H, W], bf16, kind="ExternalOutput").ap()

    with tile.TileContext(nc) as tc:
        with (
            tc.tile_pool(name="consts", bufs=1) as consts,
            tc.tile_pool(name="xpool", bufs=2) as xpool,
            tc.tile_pool(name="opool", bufs=2) as opool,
            tc.tile_pool(name="epool", bufs=4) as epool,
            tc.tile_pool(name="pspool", bufs=4, space="PSUM") as pspool,
            tc.tile_pool(name="fixpool", bufs=1, space="PSUM") as fixpool,
            tc.tile_pool(name="psum_t", bufs=1, space="PSUM") as psum_t,
        ):
            ident = consts.tile([128, 128], bf16, tag="ident")
            lhsT = [
                consts.tile([128, 128], bf16, tag=f"lhsT{t}", name=f"lhsT{t}")
                for t in range(9)
            ]
            scale_vec = consts.tile([128, 1], f32, tag="scale_vec")
            bias_vec = consts.tile([128, 1], f32, tag="bias_vec")

            # the identity (for weight transposes) must be FIRST on the Q7 queue:
            # everything queued behind the image-chunk descriptor generation
            # arrives ~10us late otherwise (memset part goes to the idle DVE)
            nc.vector.memset(ident[:], 0)
            make_identity(nc, ident[:], nomemset=True)

            # ---- weight DMAs: SWDGE bf16 cast, queued on Q7 right after the
            # identity and ahead of the image chunks (~0.6 MB, lands ~10us) ----
            # [128, 1152] layout: partition (h,o) holds W[o, i=32h..32h+31, :, :, :]
            wp = consts.tile([128, 32 * 9 * NB], bf16, tag="wp")
            wn = consts.tile([128, 32 * 9 * NB], bf16, tag="wn")
            pw_f = pw_d.rearrange("o i kh kw b -> o (i kh kw b)")
            nw_f = nw_d.rearrange("o i kh kw b -> o (i kh kw b)")
            nc.gpsimd.dma_start(wp[0:C, :], pw_f[:, 0:1152])
            nc.gpsimd.dma_start(wp[C:128, :], pw_f[:, 1152:2304])
            nc.gpsimd.dma_start(wn[0:C, :], nw_f[:, 0:1152])
            nc.gpsimd.dma_start(wn[C:128, :], nw_f[:, 1152:2304])
            pbt = consts.tile([128, NB], f32, tag="pbt")
            nbt = consts.tile([128, NB], f32, tag="nbt")
            nc.sync.dma_start(pbt[0:C, :], pb_d)
            nc.sync.dma_start(pbt[C:128, :], pb_d)
            nc.sync.dma_start(nbt[0:C, :], nb_d)
            nc.sync.dma_start(nbt[C:128, :], nb_d)
            bsv = consts.tile([128, 1], f32, tag="bsv")
            nc.sync.dma_start(bsv[:], bs_d.to_broadcast((128, 1)))
            nc.sync.dma_start(scale_vec[:], sc_d.to_broadcast((128, 1)))

            # ---- image load pipeline: chunked SWDGE cast-DMAs (f32 -> bf16) ----
            # fine chunks for image 0 (compute chases the stream); coarse after
            # (Q7 descriptor-gen is ~1us per call). DMAs before memsets: the
            # ranges are disjoint, and the DMAs are the critical path.
            def load_image(b):
                xs = xpool.tile([128, XCOLS], bf16, tag="xs", name=f"xs{b}", bufs=3)
                v0 = xs[0:C, XBASE + W : XBASE + W + 57 * W].rearrange(
                    "p (r w) -> p r w", w=W
                )
                v1 = xs[C:128, XBASE : XBASE + 57 * W].rearrange(
                    "p (r w) -> p r w", w=W
                )
                # memsets FIRST: on the in-order Q7 queue they carry the
                # buffer-reuse (WAR) gate for the chunk DMAs behind them
                nc.gpsimd.memset(xs[0:C, 0 : XBASE + W], 0)
                nc.gpsimd.memset(xs[0:C, XCOLS - 1 : XCOLS], 0)
                nc.gpsimd.memset(xs[C:128, 0:XBASE], 0)
                nc.gpsimd.memset(xs[C:128, XBASE + 57 * W : XCOLS], 0)
                chunks = IN_CHUNKS if b == 0 else [(0, 29), (29, 28)]
                for r0, nr in chunks:
                    nc.gpsimd.dma_start(v0[:, r0 : r0 + nr, :], x_d[b, :, r0 : r0 + nr, :])
                    nc.gpsimd.dma_start(
                        v1[:, r0 : r0 + nr, :], x_d[b, :, 55 + r0 : 55 + r0 + nr, :]
                    )
                return xs

            xs_next = load_image(0)
            xs_next2 = load_image(1)

            # ---- weight/bias reconstruction (overlaps image-0 DMA) ----
            # Processed in 3 transpose-groups (taps 0-3 / 4-7 / 8) so lhsT[0]
            # is ready early and the first conv matmuls start while the rest
            # of the weight prep continues.
            nc.vector.tensor_sub(wp[:], wp[:], wn[:])  # d = p - n
            # bit-combine into tap-major W_int [(h,o), (t, il)]:
            # w = ((d0*8 + d3) + d1*4) + d2*2 via scalar_tensor_tensor chains
            # (f32 in, bf16 out on the last write; integers <=15 stay exact)
            d_v = wp[:].rearrange("p (i t b) -> p t i b", t=9, b=NB)
            for t in range(9):
                nc.vector.memset(lhsT[t][:], 0)
            groups = [(0, 4), (4, 4), (8, 1)]
            tps_l = []
            for g, (tp0, ntp) in enumerate(groups):
                # separate per-group tiles keep the transpose's dependency
                # exactly on this group's STT chain
                cols = 32 * ntp
                wig = consts.tile([128, cols], bf16, tag=f"wi{g}", name=f"wi{g}")
                wtg = consts.tile([128, cols], bf16, tag=f"wt{g}", name=f"wt{g}")
                dv = d_v[:, tp0 : tp0 + ntp]
                wiv = wig[:].rearrange("p (t i) -> p t i", t=ntp)
                wtv = wtg[:].rearrange("p (t i) -> p t i", t=ntp)
                nc.vector.scalar_tensor_tensor(
                    out=wtv, in0=dv[:, :, :, 0], scalar=8.0, in1=dv[:, :, :, 3],
                    op0=mult, op1=add,
                )
                nc.vector.scalar_tensor_tensor(
                    out=wiv, in0=dv[:, :, :, 1], scalar=4.0, in1=wtv,
                    op0=mult, op1=add,
                )
                nc.vector.scalar_tensor_tensor(
                    out=wtv, in0=dv[:, :, :, 2], scalar=2.0, in1=wiv,
                    op0=mult, op1=add,
                )
                tps = psum_t.tile([128, 128], bf16, tag="tps", name=f"tps{g}", bufs=3)
                nc.tensor.transpose(tps[0:cols, :], wtg[:], ident[:])
                tps_l.append(tps)
            for g, (tp0, ntp) in enumerate(groups):
                tps = tps_l[g]
                for t in range(tp0, tp0 + ntp):
                    rt = (t - tp0) * 32
                    nc.scalar.copy(lhsT[t][0:32, 0:C], tps[rt : rt + 32, 0:C])
                    nc.vector.tensor_copy(
                        lhsT[t][32:C, 0:C], tps[rt : rt + 32, C:128]
                    )
                    nc.vector.tensor_copy(
                        lhsT[t][C : C + 32, C:128], tps[rt : rt + 32, 0:C]
                    )
                    nc.vector.tensor_copy(
                        lhsT[t][C + 32 : 128, C:128], tps[rt : rt + 32, C:128]
                    )
            # bias vector, duplicated across both partition blocks
            nc.vector.tensor_sub(pbt[:], pbt[:], nbt[:])
            btmp = consts.tile([128, 1], f32, tag="btmp")
            nc.vector.scalar_tensor_tensor(
                out=btmp[:], in0=pbt[:, 0:1], scalar=8.0, in1=pbt[:, 3:4],
                op0=mult, op1=add,
            )
            nc.vector.scalar_tensor_tensor(
                out=bias_vec[:], in0=pbt[:, 1:2], scalar=4.0, in1=btmp[:],
                op0=mult, op1=add,
            )
            nc.vector.scalar_tensor_tensor(
                out=btmp[:], in0=pbt[:, 2:3], scalar=2.0, in1=bias_vec[:],
                op0=mult, op1=add,
            )
            nc.vector.tensor_mul(btmp[:], btmp[:], bsv[:])
            nc.scalar.mul(bias_vec[:], btmp[:], 1.0 / 15.0)
            nc.scalar.mul(scale_vec[:], scale_vec[:], 1.0 / 15.0)

            # ---- main conv loop ----
            for b in range(BPC):
                xs = xs_next
                xs_next = xs_next2
                xs_next2 = load_image(b + 2) if b + 2 < BPC else None

                outb = opool.tile([128, GW], bf16, tag="outb")
                ov = outb[:].rearrange("p (r w) -> p r w", w=W)
                last = b == BPC - 1

                def emit_tiles(tiles, xs=xs, outb=outb):
                    for n0, nt in tiles:
                        ps = pspool.tile([128, 512], f32, tag="ps")
                        for t, off in enumerate(TAP_OFFS):
                            nc.tensor.matmul(
                                ps[:, 0:nt],
                                lhsT[t][:],
                                xs[:, XBASE + n0 + off : XBASE + n0 + off + nt],
                                start=(t == 0),
                                stop=(t == 8),
                            )
                        nc.scalar.activation(
                            outb[:, n0 : n0 + nt],
                            ps[:, 0:nt],
                            mybir.ActivationFunctionType.Identity,
                            bias=bias_vec[:],
                            scale=scale_vec[:],
                        )

                def emit_gathers(xs=xs):
                    # edge sequences (wrap contamination sources), pre-scaled
                    xv0 = xs[0:C, XBASE + W : XBASE + 58 * W].rearrange(
                        "p (r w) -> p r w", w=W
                    )
                    xv1 = xs[C:128, XBASE : XBASE + 57 * W].rearrange(
                        "p (r w) -> p r w", w=W
                    )
                    eR = epool.tile([128, 58], bf16, tag="eR", name=f"eR{b}")
                    eL = epool.tile([128, 58], bf16, tag="eL", name=f"eL{b}")
                    nc.gpsimd.memset(eR[:], 0)
                    nc.gpsimd.memset(eL[:], 0)
                    nc.vector.tensor_scalar_mul(
                        eR[0:C, 2:58], xv0[:, 0:56, W - 1], scale_vec[0:C]
                    )
                    nc.vector.tensor_scalar_mul(
                        eR[C:128, 1:58], xv1[:, 0:57, W - 1], scale_vec[C:128]
                    )
                    nc.vector.tensor_scalar_mul(
                        eL[0:C, 0:57], xv0[:, 0:57, 0], scale_vec[0:C]
                    )
                    nc.vector.tensor_scalar_mul(
                        eL[C:128, 0:56], xv1[:, 1:57, 0], scale_vec[C:128]
                    )
                    return eR, eL

                def emit_fix_out(psF, eR, eL, qs, qn, b=b, ov=ov, outb=outb):
                    # repair output columns 0 and 111, then stream the rows out
                    for dr in range(3):
                        nc.tensor.matmul(
                            psF[:, qs : qs + qn],
                            lhsT[dr * 3 + 0][:],
                            eR[:, dr + qs : dr + qs + qn],
                            start=(dr == 0),
                            stop=(dr == 2),
                        )
                    for dr in range(3):
                        nc.tensor.matmul(
                            psF[:, 64 + qs : 64 + qs + qn],
                            lhsT[dr * 3 + 2][:],
                            eL[:, dr + qs : dr + qs + qn],
                            start=(dr == 0),
                            stop=(dr == 2),
                        )
                    nc.vector.tensor_sub(
                        ov[:, qs : qs + qn, 0],
                        ov[:, qs : qs + qn, 0],
                        psF[:, qs : qs + qn],
                    )
                    nc.vector.tensor_sub(
                        ov[:, qs : qs + qn, W - 1],
                        ov[:, qs : qs + qn, W - 1],
                        psF[:, 64 + qs : 64 + qs + qn],
                    )
                    # top group on the sync HWDGE ring, bottom on the scalar ring
                    nc.sync.dma_start(
                        y_d[b, :, qs : qs + qn, :],
                        outb[0:C, qs * W : (qs + qn) * W],
                    )
                    nc.scalar.dma_start(
                        y_d[b, :, HALF + qs : HALF + qs + qn, :],
                        outb[C:128, qs * W : (qs + qn) * W],
                    )

                psF = fixpool.tile([128, 512], f32, tag="psF")
                if last:
                    # interleave fix+writeback with the remaining tiles so only
                    # the last 14 rows' transfer (0.4 MB) sits in the tail
                    emit_tiles(N_TILES[:7])
                    eR, eL = emit_gathers()
                    emit_fix_out(psF, eR, eL, 0, 28)
                    emit_tiles(N_TILES[7:10])
                    emit_fix_out(psF, eR, eL, 28, 14)
                    emit_tiles(N_TILES[10:])
                    emit_fix_out(psF, eR, eL, 42, 14)
                else:
                    emit_tiles(N_TILES)
                    eR, eL = emit_gathers()
                    emit_fix_out(psF, eR, eL, 0, 56)

    nc.compile()
    _CACHE["nc"] = nc
    return nc


def _run(inputs, trace=False):
    from concourse.bass_utils import run_bass_kernel_spmd

    nc = _build()
    x = np.ascontiguousarray(np.asarray(inputs["x"], dtype=np.float32))
    shared = {
        "pweight": np.ascontiguousarray(np.asarray(inputs["pweight"], np.float32)),
        "nweight": np.ascontiguousarray(np.asarray(inputs["nweight"], np.float32)),
        "scale": np.ascontiguousarray(np.asarray(inputs["scale"], np.float32)),
        "pbias": np.ascontiguousarray(np.asarray(inputs["pbias"], np.float32)),
        "nbias": np.ascontiguousarray(np.asarray(inputs["nbias"], np.float32)),
        "biasscale": np.ascontiguousarray(np.asarray(inputs["biasscale"], np.float32)),
    }
    in_maps = [dict(shared, x=x[c * BPC : (c + 1) * BPC]) for c in range(CORES)]
    last_err = None
    for attempt in range(3):
        try:
            res = run_bass_kernel_spmd(
                nc, in_maps, core_ids=list(range(CORES)), trace=trace
            )
            out = np.concatenate(
                [np.asarray(res.results[c]["y"]) for c in range(CORES)], axis=0
            ).astype(np.float32)
            return out, res.exec_time_ns
        except Exception as e:  # transient NRT_EXEC_UNIT_UNRECOVERABLE recovers on retry
            last_err = e
            import time

            time.sleep(10)
    raise last_err


def kernel(**inputs) -> np.ndarray:
    out, _ = _run(inputs)
    return out
